# revision 42
# baseline (speedup 1.0000x reference)
"""AdaptiveUnpooling (GNN message passing) on 8 TRN2 NeuronCores.

Strategy:
  - Host: build undirected edge list, lexsort by (tgt, src), dedup, drop
    self-loops.  Shard edges by *target range* (no collectives needed:
    each core owns a contiguous slice of output rows).
  - Only edges with a MISSING target and a PRESENT source are gathered
    (~40% of all unique edges): present targets keep x0 verbatim (the
    epilogue mask a = missing/max(cnt,1) is 0 there) and missing-source
    edges are count-only bookkeeping folded into `a` on the host.  This
    halves the SWDGE descriptor load vs gathering every edge.
  - Device (per core): dma_gather source-feature rows from the HBM-resident
    feature table (bf16, channel-padded to 256B rows); build one-hot
    (edge -> local target) matrices on the vector engine (bf16); TensorE
    matmuls accumulate per-128-target-window feature sums in PSUM;
    epilogue computes  out = feat * a + x0 * (1 - missing)  per window,
    which reproduces  where(missing & cnt>0, feat_sum/cnt, x0)  exactly.
  - dma_gather indices are int16, so the table is gathered in two halves
    (rows < 32768 and >= 32768), each a flat window-major tile stream.
  - SWDGE economics (measured): aggregate descriptor throughput caps at
    ~2.4ns/row shared across all 4 queues, at most ~2 gather instructions
    make progress concurrently, each instruction has a ~4.4us floor
    (drain-paced completion), and calls above ~1800 descriptors hit an
    outstanding-descriptor stall (8ns/row).  Gather calls are therefore
    grouped as one call per (table-half, 2-window group) -- ~800-1500
    rows per call -- rotated over the 4 SWDGE queues, double-buffered
    through per-stream staging rings.
  - The epilogue PSUM read (feat * a) runs on the otherwise-idle
    Activation engine (scalar.activation Copy with per-partition scale),
    keeping DVE for the one-hot build and the x0 add only.  The idx
    array load is split so the first gather groups start ~15us earlier.
"""
import numpy as np
import ml_dtypes

BF16 = ml_dtypes.bfloat16
W = 128            # targets per window (= PSUM partition dim)
CP = 128           # channel-padded table row (bf16 -> 256B)
HALF = 32768       # int16 index limit for dma_gather
PAD_TLOC = -1000.0
NEG_PAD = False    # -1 trailing pads desync the SWDGE ring bookkeeping on HW; keep 0-pads
NQUEUES = 4        # SWDGE queues to spread gather desc-gen over
PSUM_BUFS = 8
import os as _os
CHUNK_T = int(_os.environ.get("K_CHUNK_T", "2"))    # windows per gather call group
SCRATCH = int(_os.environ.get("K_SCRATCH", "16384"))  # SWDGE desc carveout bytes/partition

LAST_EXEC_NS = None
LAST_RESULTS = None


def _prep(x_abstract, perm, edge_index, N, n_cores):
    """Host-side index preprocessing. Returns per-core input arrays + schedule."""
    NP, C = x_abstract.shape
    perm = np.asarray(perm).astype(np.int64)
    e = np.asarray(edge_index).astype(np.int64)

    tgt = np.concatenate([e[0], e[1]])
    src = np.concatenate([e[1], e[0]])
    order = np.lexsort((src, tgt))
    t_s = tgt[order]
    s_s = src[order]
    uniq = np.empty(t_s.shape, dtype=bool)
    uniq[0] = True
    uniq[1:] = (t_s[1:] != t_s[:-1]) | (s_s[1:] != s_s[:-1])
    keep = uniq & (t_s != s_s)
    t_u = t_s[keep]
    s_u = s_s[keep]                      # sorted by (t, s)

    inv = np.full(N, -1, np.int64)
    inv[perm] = np.arange(NP)
    missing = np.ones(N, bool)
    missing[perm] = False

    NWIN = ((N + n_cores - 1) // n_cores + W - 1) // W   # ceil(ceil(N/n_cores)/W)
    TPC = NWIN * W                       # targets per core (padded)

    # Only edges with a MISSING target and a PRESENT source need feature
    # gathers: present targets keep x0 (the epilogue multiplies feat by
    # a = missing/max(cnt,1) which is 0 there), and missing-source edges
    # are count-only bookkeeping folded into `a` on the host.
    sidx_full = inv[s_u]
    gmask = missing[t_u] & (sidx_full >= 0)
    t_g = t_u[gmask]
    sidx = sidx_full[gmask]              # table row of source (>= 0)
    core = t_g // TPC
    tl = t_g - core * TPC                # target local to core
    win = tl // W
    j = tl % W                           # local target within window

    # group: 0 = present half A, 1 = present half B
    grp = np.where(sidx < HALF, 0, 1)

    key = (core * NWIN + win) * 2 + grp
    cnts = np.bincount(key, minlength=n_cores * NWIN * 2).reshape(n_cores, NWIN, 2)
    nA = cnts[:, :, 0]
    nB = cnts[:, :, 1]

    nAmax = np.maximum.reduce(nA, axis=0)
    nBmax = np.maximum.reduce(nB, axis=0)
    TFA = -(-nAmax // 128)               # tiles, max over cores
    TFB = -(-nBmax // 128)
    # ensure at least one gathered (feature) tile per window so the PSUM
    # feature region is always written (0 * garbage could be NaN otherwise)
    for w in range(NWIN):
        if TFA[w] + TFB[w] == 0:
            TFA[w] = 1
            nAmax[w] = 16

    BT = TFA + TFB                       # one-hot tiles per window (feature only)
    # flat per-half tile streams (window-major); gather calls are CHUNK_T-tile
    # chunks of each stream, decoupled from windows (amortizes the ~3.6us
    # fixed cost per SWDGE gather instruction)
    a_off = np.concatenate([[0], np.cumsum(TFA)])   # A-stream tile offset per window
    b_off = np.concatenate([[0], np.cumsum(TFB)])
    TA = int(a_off[-1])                  # total A tiles
    TB = int(b_off[-1])
    t_off = np.concatenate([[0], np.cumsum(BT)])    # tloc cols
    SBT = int(t_off[-1])                 # total tloc columns
    NIDXC = (TA + TB) * 8                # idx cols: A-stream then B-stream

    gidx = np.zeros((n_cores, 128, NIDXC), np.int16)
    tloc = np.full((n_cores, 128, SBT), PAD_TLOC, np.float32)

    skey = np.lexsort((grp, win, core))
    c2, w2, g2, j2, sx2 = core[skey], win[skey], grp[skey], j[skey], sidx[skey]
    bkey = (c2 * NWIN + w2) * 2 + g2
    bounds = np.searchsorted(bkey, np.arange(n_cores * NWIN * 2 + 1))
    for c in range(n_cores):
        for w in range(NWIN):
            base = (c * NWIN + w) * 2
            toff = int(t_off[w])
            for g in range(2):
                lo, hi = bounds[base + g], bounds[base + g + 1]
                n = hi - lo
                if n == 0:
                    continue
                i = np.arange(n)
                jj = j2[lo:hi].astype(np.float32)
                if g == 0:
                    tile0 = 0
                    coff = int(a_off[w]) * 8
                    gidx[c, i % 16, coff + i // 16] = sx2[lo:hi]
                else:
                    tile0 = int(TFA[w])
                    coff = (TA + int(b_off[w])) * 8
                    gidx[c, i % 16, coff + i // 16] = sx2[lo:hi] - HALF
                tloc[c, i % 128, toff + tile0 + i // 128] = jj
    gidx[:, 16:, :] = np.tile(gidx[:, :16, :], (1, 7, 1))

    # mmask / x0m  (x0 * (1-missing)), per-core window-major layout
    x0m_full = np.zeros((n_cores * TPC, C), np.float32)
    x0m_full[perm] = np.asarray(x_abstract, np.float32)
    x0m = (
        x0m_full.reshape(n_cores, NWIN, W, C)
        .transpose(0, 2, 1, 3)
        .reshape(n_cores, 128, NWIN * C)
        .copy()
    )
    cnt_full = np.bincount(t_u, minlength=N).astype(np.float32)
    a_full = np.zeros(n_cores * TPC, np.float32)
    a_full[:N] = missing.astype(np.float32) / np.maximum(cnt_full, 1.0)
    mmask = (
        a_full.reshape(n_cores, NWIN, W).transpose(0, 2, 1).reshape(n_cores, 128, NWIN).copy()
    )

    iota = np.broadcast_to(np.arange(W, dtype=np.float32), (128, W)).astype(BF16).copy()
    tloc_bf = tloc.astype(BF16)

    sched = dict(
        NWIN=NWIN, TPC=TPC, C=C, NP=NP,
        TFA=[int(x) for x in TFA], TFB=[int(x) for x in TFB],
        BT=[int(x) for x in BT],
        a_off=[int(x) for x in a_off], b_off=[int(x) for x in b_off],
        TA=TA, TB=TB,
        t_off=[int(x) for x in t_off],
        NIDXC=NIDXC, SBT=SBT,
    )
    arrays = dict(gidx=gidx, tloc=tloc_bf, x0m=x0m, mmask=mmask, iota=iota)
    return sched, arrays


def _model_numpy(table, sched, arrays, n_cores):
    """Numpy replica of the device computation (for validating prep)."""
    NWIN, C = sched["NWIN"], sched["C"]
    TFA, TFB = sched["TFA"], sched["TFB"]
    g_off, t_off = sched["g_off"], sched["t_off"]
    NP = sched["NP"]
    tb = np.asarray(table, np.float32).astype(BF16).astype(np.float32)
    outs = []
    for c in range(n_cores):
        gidx = arrays["gidx"][c]
        tloc = np.asarray(arrays["tloc"][c], np.float32)
        x0m = arrays["x0m"][c]
        mm = arrays["mmask"][c]
        out = np.zeros((NWIN * W, C), np.float32)
        for w in range(NWIN):
            ntf = TFA[w] + TFB[w]
            bt = ntf
            stag = np.zeros((128, ntf, C), np.float32)
            for half, (nt, coff, base) in enumerate(
                [(TFA[w], g_off[w], 0), (TFB[w], g_off[w] + TFA[w] * 8, HALF)]
            ):
                ni = nt * 128
                if ni == 0:
                    continue
                i = np.arange(ni)
                idx = gidx[i % 16, coff + i // 16].astype(np.int64)
                rows = tb[np.clip(idx + base, 0, NP - 1)]
                t0 = 0 if half == 0 else TFA[w]
                stag[i % 128, t0 + i // 128] = rows
            tl = tloc[:, t_off[w]:t_off[w] + bt]
            oh = (np.arange(W)[None, None, :] == tl[:, :, None]).astype(np.float32)
            feat = np.zeros((W, C), np.float32)
            for t in range(bt):
                feat += oh[:, t, :].T @ stag[:, t, :]
            a = mm[:, w]
            out[w * W:(w + 1) * W] = feat * a[:, None] + x0m[:, w * C:(w + 1) * C]
        outs.append(out)
    return outs


def _build_nc(sched):
    import concourse.bacc as bacc
    import concourse.mybir as mybir
    from concourse import tile

    NWIN, C, NP = sched["NWIN"], sched["C"], sched["NP"]
    TFA, TFB, BT = sched["TFA"], sched["TFB"], sched["BT"]
    a_off, b_off = sched["a_off"], sched["b_off"]
    TA, TB = sched["TA"], sched["TB"]
    t_off = sched["t_off"]
    NIDXC, SBT = sched["NIDXC"], sched["SBT"]
    MAXBT = max(BT)
    f32 = mybir.dt.float32
    bf16 = mybir.dt.bfloat16

    nc = bacc.Bacc(None, num_swdge_queues=NQUEUES, dynamic_dma_scratch_size=SCRATCH)
    table_d = nc.dram_tensor("table", [NP, CP], bf16, kind="ExternalInput")
    gidx_d = nc.dram_tensor("gidx", [128, NIDXC], mybir.dt.int16, kind="ExternalInput")
    tloc_d = nc.dram_tensor("tloc", [128, SBT], bf16, kind="ExternalInput")
    iota_d = nc.dram_tensor("iota", [128, W], bf16, kind="ExternalInput")
    mm_d = nc.dram_tensor("mmask", [128, NWIN], f32, kind="ExternalInput")
    x0m_d = nc.dram_tensor("x0m", [128, NWIN * C], f32, kind="ExternalInput")
    out_d = nc.dram_tensor("out", [NWIN * W, C], f32, kind="ExternalOutput")

    tabA = table_d[0:min(HALF, NP), :]
    tabB = table_d[HALF:NP, :] if NP > HALF else None

    # gather-call plan: one call per (stream, GRP-window group). Keeps calls
    # in the ~800-1600 desc sweet spot: big enough to amortize the ~4.4us
    # per-call floor (drain-paced slice + in-flight-2 dispatch), small enough
    # to avoid the outstanding-descriptor stall seen at ~2k+ desc calls.
    # B-half windows carry ~half the tiles of A, so B groups can span more
    # windows for the same call size.
    GRPA = CHUNK_T
    GRPB = int(_os.environ.get("K_GRPB", str(CHUNK_T)))
    GRPS = (GRPA, GRPB)

    def plan(offs, stream):
        chunks = []
        grp = GRPS[stream]
        for w0 in range(0, NWIN, grp):
            w1 = min(w0 + grp, NWIN)
            t0, t1 = offs[w0], offs[w1]
            if t1 > t0:
                chunks.append((stream, t0, t1 - t0, w0))
        return chunks

    # interleave A/B chunk emissions in window order of first use
    emits = {w: [] for w in range(NWIN)}
    for stream, t0, nt, w0 in plan(a_off, 0) + plan(b_off, 1):
        emits[w0].append((stream, t0, nt))

    DEP = int(_os.environ.get("K_DEP", "3"))  # in-flight chunk slots per stream
    qn = [0]

    def next_q(stream):
        q = qn[0] % NQUEUES
        qn[0] += 1
        return q

    with tile.TileContext(nc) as tc:
        with (
            tc.tile_pool(name="const", bufs=1) as cpool,
            tc.tile_pool(name="oh", bufs=4) as opool,
            tc.tile_pool(name="psum", bufs=PSUM_BUFS, space="PSUM") as ppool,
            tc.tile_pool(name="outb", bufs=4) as bpool,
        ):
            GA_MAX = max(a_off[min(w0 + GRPA, NWIN)] - a_off[w0] for w0 in range(0, NWIN, GRPA))
            GB_MAX = max(b_off[min(w0 + GRPB, NWIN)] - b_off[w0] for w0 in range(0, NWIN, GRPB))
            idx_s = cpool.tile([128, NIDXC], mybir.dt.int16)
            tloc_s = cpool.tile([128, SBT], bf16)
            iota_s = cpool.tile([128, W], bf16)
            m_s = cpool.tile([128, NWIN], f32)
            x0m_s = cpool.tile([128, NWIN * C], f32)
            stagA = cpool.tile([128, DEP * GA_MAX * CP], bf16)
            stagB = cpool.tile([128, DEP * max(GB_MAX, 1) * CP], bf16)
            stA = stagA[:].rearrange("p (t c) -> p t c", c=CP)
            stB = stagB[:].rearrange("p (t c) -> p t c", c=CP)
            # split the idx load so the first gather groups' indices land
            # quickly instead of waiting for the full 1.6MB array
            wE = min(4 * GRPA, NWIN)
            cutA = a_off[wE] * 8
            cutB = (TA + b_off[min(4 * GRPB, NWIN)]) * 8
            nc.sync.dma_start(idx_s[:, 0:cutA], gidx_d[:, 0:cutA])
            nc.sync.dma_start(idx_s[:, TA * 8:cutB], gidx_d[:, TA * 8:cutB])
            nc.sync.dma_start(idx_s[:, cutA:TA * 8], gidx_d[:, cutA:TA * 8])
            nc.sync.dma_start(idx_s[:, cutB:NIDXC], gidx_d[:, cutB:NIDXC])
            nc.sync.dma_start(iota_s[:], iota_d[:])
            nc.sync.dma_start(tloc_s[:], tloc_d[:])
            nc.sync.dma_start(m_s[:], mm_d[:])
            nc.sync.dma_start(x0m_s[:], x0m_d[:])
            dummy = None
            if _os.environ.get("K_DIAG_NOCONS"):
                dummy = cpool.tile([128, CP], bf16)
                nc.vector.memset(dummy[:], 0.0)

            def slot_ap(stream, w, g):
                # g = global tile index in the stream; slot by window group
                st = stA if stream == 0 else stB
                gmax = GA_MAX if stream == 0 else max(GB_MAX, 1)
                offs = a_off if stream == 0 else b_off
                grp = GRPS[stream]
                grpi = w // grp
                base = ((grpi % DEP) * gmax) + (g - offs[grpi * grp])
                return st[:, base, :]

            for w in range(NWIN):
                for stream, t0, nt in emits[w]:
                    ni = nt * 128
                    tab = tabA if stream == 0 else tabB
                    coff = t0 * 8 if stream == 0 else (TA + t0) * 8
                    st = stA if stream == 0 else stB
                    gmax = GA_MAX if stream == 0 else max(GB_MAX, 1)
                    base = ((w // GRPS[stream]) % DEP) * gmax
                    nc.gpsimd.dma_gather(
                        st[:, base:base + nt, :], tab,
                        idx_s[:, coff:coff + ni // 16],
                        ni, ni, CP, single_packet=False, queue_num=next_q(stream),
                    )
                bt = BT[w]
                oh = opool.tile([128, MAXBT * W], bf16, tag="oh")
                oh3 = oh[:].rearrange("p (t w) -> p t w", w=W)
                nc.vector.tensor_tensor(
                    oh3[:, 0:bt, :],
                    iota_s[:].unsqueeze(1).broadcast_to([128, bt, W]),
                    tloc_s[:, t_off[w]:t_off[w] + bt].unsqueeze(2).broadcast_to([128, bt, W]),
                    mybir.AluOpType.is_equal,
                )
                psum = ppool.tile([128, C], f32, tag="ps")
                for t in range(bt):
                    if dummy is not None:
                        stile = dummy[:]
                    elif t < TFA[w]:
                        stile = slot_ap(0, w, a_off[w] + t)
                    else:
                        stile = slot_ap(1, w, b_off[w] + t - TFA[w])
                    nc.tensor.matmul(
                        psum[:, 0:C], oh3[:, t, :], stile[:, 0:C],
                        start=(t == 0), stop=(t == bt - 1), skip_group_check=True,
                    )
                outb = bpool.tile([128, C], f32, tag="outb")
                if _os.environ.get("K_EPI_DVE"):
                    nc.vector.tensor_scalar(
                        outb[:], psum[:, 0:C], m_s[:, w:w + 1], None, mybir.AluOpType.mult
                    )
                else:
                    # PSUM read on the (otherwise idle) Activation engine:
                    # outb = psum * a_w
                    nc.scalar.activation(
                        outb[:], psum[:, 0:C], mybir.ActivationFunctionType.Copy,
                        scale=m_s[:, w:w + 1],
                    )
                nc.vector.tensor_tensor(
                    outb[:], outb[:], x0m_s[:, w * C:(w + 1) * C], mybir.AluOpType.add
                )
                nc.sync.dma_start(out_d[w * W:(w + 1) * W, :], outb[:])
    return nc


def _register_ntff_hook():
    """Provide antenv.axon_hooks (absent in this image) so trace=True works."""
    import sys
    import types
    import ctypes
    import contextlib

    try:
        import antenv.axon_hooks  # noqa: F401
        return True
    except ImportError:
        pass
    so_path = "/opt/axon/libaxon_pjrt.so"
    try:
        lib = ctypes.CDLL(so_path)
    except OSError:
        return False
    if not hasattr(lib, "axon_start_nrt_profile"):
        return False
    lib.axon_start_nrt_profile.argtypes = [
        ctypes.POINTER(ctypes.c_int64),
        ctypes.c_size_t,
    ]
    lib.axon_start_nrt_profile.restype = ctypes.c_int64
    lib.axon_stop_nrt_profile.argtypes = [ctypes.c_char_p]
    lib.axon_stop_nrt_profile.restype = ctypes.c_int64

    @contextlib.contextmanager
    def _hook(output_dir, device_ids):
        import jax

        jax.devices()
        if device_ids:
            ids = (ctypes.c_int64 * len(device_ids))(*device_ids)
            rc = lib.axon_start_nrt_profile(ids, len(device_ids))
        else:
            rc = lib.axon_start_nrt_profile(None, 0)
        if rc != 0:
            raise RuntimeError(f"axon_start_nrt_profile rc={rc}")
        try:
            yield
        finally:
            lib.axon_stop_nrt_profile(str(output_dir).encode())

    mod = types.ModuleType("antenv.axon_hooks")
    mod.get_axon_ntff_profile_hook = lambda: _hook
    mod.set_axon_ntff_profile_hook = lambda h: None
    sys.modules["antenv.axon_hooks"] = mod
    return True


def kernel(x_abstract, perm, edge_index, original_num_nodes):
    global LAST_EXEC_NS, LAST_RESULTS
    import os
    from concourse import bass_utils
    from concourse.bass_utils import run_bass_kernel_spmd

    N = int(original_num_nodes)
    n_cores = 8
    x_abstract = np.ascontiguousarray(np.asarray(x_abstract, np.float32))
    sched, arrays = _prep(x_abstract, perm, edge_index, N, n_cores)

    NP = sched["NP"]
    table_bf = np.zeros((NP, CP), BF16)
    table_bf[:, :x_abstract.shape[1]] = x_abstract.astype(BF16)

    nc = _build_nc(sched)
    nc.finalize()

    in_maps = []
    for c in range(n_cores):
        in_maps.append(
            dict(
                table=table_bf,
                gidx=arrays["gidx"][c],
                tloc=arrays["tloc"][c],
                iota=arrays["iota"],
                mmask=arrays["mmask"][c],
                x0m=arrays["x0m"][c],
            )
        )
    trace = bool(int(os.environ.get("KERNEL_TRACE", "0")))
    if trace:
        trace = _register_ntff_hook()
        bass_utils.upload_artifacts = lambda tmpdir: f"local:{tmpdir}"
    try:
        res = run_bass_kernel_spmd(
            nc, in_maps, core_ids=list(range(n_cores)), trace=trace
        )
    except Exception:
        if not trace:
            raise
        res = run_bass_kernel_spmd(
            nc, in_maps, core_ids=list(range(n_cores)), trace=False
        )
    LAST_RESULTS = res
    LAST_EXEC_NS = getattr(res, "exec_time_ns", None)
    out = np.concatenate([res.results[c]["out"] for c in range(n_cores)], axis=0)
    return out[:N]



# revision 48
# speedup vs baseline: 1.0410x; 1.0410x over previous
"""AdaptiveUnpooling (GNN message passing) on 8 TRN2 NeuronCores.

Strategy:
  - Host: build undirected edge list, lexsort by (tgt, src), dedup, drop
    self-loops.  Shard edges by *target range* (no collectives needed:
    each core owns a contiguous slice of output rows).
  - Only edges with a MISSING target and a PRESENT source are gathered
    (~40% of all unique edges): present targets keep x0 verbatim (the
    epilogue mask a = missing/max(cnt,1) is 0 there) and missing-source
    edges are count-only bookkeeping folded into `a` on the host.  This
    halves the SWDGE descriptor load vs gathering every edge.
  - Device (per core): dma_gather source-feature rows from the HBM-resident
    feature table (bf16, channel-padded to 256B rows); build one-hot
    (edge -> local target) matrices on the vector engine (bf16); TensorE
    matmuls accumulate per-128-target-window feature sums in PSUM;
    epilogue computes  out = feat * a + x0 * (1 - missing)  per window,
    which reproduces  where(missing & cnt>0, feat_sum/cnt, x0)  exactly.
  - dma_gather indices are int16, so the table is gathered in two halves
    (rows < 32768 and >= 32768), each a flat window-major tile stream.
  - SWDGE economics (measured): aggregate descriptor throughput caps at
    ~2.4ns/row shared across all 4 queues, at most ~2 gather instructions
    make progress concurrently, each instruction has a ~4.4us floor
    (drain-paced completion), and calls above ~1800 descriptors hit an
    outstanding-descriptor stall (8ns/row).  Gather calls are therefore
    grouped as one call per (table-half, 2-window group) -- ~800-1500
    rows per call -- rotated over the 4 SWDGE queues, double-buffered
    through per-stream staging rings.
  - The epilogue PSUM read (feat * a) runs on the otherwise-idle
    Activation engine (scalar.activation Copy with per-partition scale),
    keeping DVE for the one-hot build and the x0 add only.  The idx
    array load is split so the first gather groups start ~15us earlier.
"""
import numpy as np
import ml_dtypes

BF16 = ml_dtypes.bfloat16
FP8 = ml_dtypes.float8_e4m3fn
import os as _os0
USE_FP8 = _os0.environ.get("K_FP8", "1") == "1"
W = 128            # targets per window (= PSUM partition dim)
CP = 256 if USE_FP8 else 128   # channel-padded table row (-> 256B either way)
HALF = 32768       # int16 index limit for dma_gather
PAD_TLOC = -1000.0
NEG_PAD = False    # -1 trailing pads desync the SWDGE ring bookkeeping on HW; keep 0-pads
NQUEUES = 4        # SWDGE queues to spread gather desc-gen over
PSUM_BUFS = 8
import os as _os
CHUNK_T = int(_os.environ.get("K_CHUNK_T", "2"))    # windows per gather call group
SCRATCH = int(_os.environ.get("K_SCRATCH", "16384"))  # SWDGE desc carveout bytes/partition

LAST_EXEC_NS = None
LAST_RESULTS = None


def _prep(x_abstract, perm, edge_index, N, n_cores):
    """Host-side index preprocessing. Returns per-core input arrays + schedule."""
    NP, C = x_abstract.shape
    perm = np.asarray(perm).astype(np.int64)
    e = np.asarray(edge_index).astype(np.int64)

    tgt = np.concatenate([e[0], e[1]])
    src = np.concatenate([e[1], e[0]])
    order = np.lexsort((src, tgt))
    t_s = tgt[order]
    s_s = src[order]
    uniq = np.empty(t_s.shape, dtype=bool)
    uniq[0] = True
    uniq[1:] = (t_s[1:] != t_s[:-1]) | (s_s[1:] != s_s[:-1])
    keep = uniq & (t_s != s_s)
    t_u = t_s[keep]
    s_u = s_s[keep]                      # sorted by (t, s)

    inv = np.full(N, -1, np.int64)
    inv[perm] = np.arange(NP)
    missing = np.ones(N, bool)
    missing[perm] = False

    NWIN = ((N + n_cores - 1) // n_cores + W - 1) // W   # ceil(ceil(N/n_cores)/W)
    TPC = NWIN * W                       # targets per core (padded)

    # Only edges with a MISSING target and a PRESENT source need feature
    # gathers: present targets keep x0 (the epilogue multiplies feat by
    # a = missing/max(cnt,1) which is 0 there), and missing-source edges
    # are count-only bookkeeping folded into `a` on the host.
    sidx_full = inv[s_u]
    gmask = missing[t_u] & (sidx_full >= 0)
    t_g = t_u[gmask]
    sidx = sidx_full[gmask]              # table row of source (>= 0)
    core = t_g // TPC
    tl = t_g - core * TPC                # target local to core
    win = tl // W
    j = tl % W                           # local target within window

    # group: 0 = present half A, 1 = present half B
    grp = np.where(sidx < HALF, 0, 1)

    key = (core * NWIN + win) * 2 + grp
    cnts = np.bincount(key, minlength=n_cores * NWIN * 2).reshape(n_cores, NWIN, 2)
    nA = cnts[:, :, 0]
    nB = cnts[:, :, 1]

    nAmax = np.maximum.reduce(nA, axis=0)
    nBmax = np.maximum.reduce(nB, axis=0)
    TFA = -(-nAmax // 128)               # tiles, max over cores
    TFB = -(-nBmax // 128)
    # ensure at least one gathered (feature) tile per window so the PSUM
    # feature region is always written (0 * garbage could be NaN otherwise)
    for w in range(NWIN):
        if TFA[w] + TFB[w] == 0:
            TFA[w] = 1
            nAmax[w] = 16

    BT = TFA + TFB                       # one-hot tiles per window (feature only)
    # flat per-half tile streams (window-major); gather calls are CHUNK_T-tile
    # chunks of each stream, decoupled from windows (amortizes the ~3.6us
    # fixed cost per SWDGE gather instruction)
    a_off = np.concatenate([[0], np.cumsum(TFA)])   # A-stream tile offset per window
    b_off = np.concatenate([[0], np.cumsum(TFB)])
    TA = int(a_off[-1])                  # total A tiles
    TB = int(b_off[-1])
    t_off = np.concatenate([[0], np.cumsum(BT)])    # tloc cols
    SBT = int(t_off[-1])                 # total tloc columns
    NIDXC = (TA + TB) * 8                # idx cols: A-stream then B-stream

    gidx = np.zeros((n_cores, 128, NIDXC), np.int16)
    tloc = np.full((n_cores, 128, SBT), PAD_TLOC, np.float32)

    skey = np.lexsort((grp, win, core))
    c2, w2, g2, j2, sx2 = core[skey], win[skey], grp[skey], j[skey], sidx[skey]
    bkey = (c2 * NWIN + w2) * 2 + g2
    bounds = np.searchsorted(bkey, np.arange(n_cores * NWIN * 2 + 1))
    for c in range(n_cores):
        for w in range(NWIN):
            base = (c * NWIN + w) * 2
            toff = int(t_off[w])
            for g in range(2):
                lo, hi = bounds[base + g], bounds[base + g + 1]
                n = hi - lo
                if n == 0:
                    continue
                i = np.arange(n)
                jj = j2[lo:hi].astype(np.float32)
                if g == 0:
                    tile0 = 0
                    coff = int(a_off[w]) * 8
                    gidx[c, i % 16, coff + i // 16] = sx2[lo:hi]
                else:
                    tile0 = int(TFA[w])
                    coff = (TA + int(b_off[w])) * 8
                    gidx[c, i % 16, coff + i // 16] = sx2[lo:hi] - HALF
                tloc[c, i % 128, toff + tile0 + i // 128] = jj
    gidx[:, 16:, :] = np.tile(gidx[:, :16, :], (1, 7, 1))

    # mmask / x0m  (x0 * (1-missing)), per-core window-major layout
    x0m_full = np.zeros((n_cores * TPC, C), np.float32)
    x0m_full[perm] = np.asarray(x_abstract, np.float32)
    x0m = (
        x0m_full.reshape(n_cores, NWIN, W, C)
        .transpose(0, 2, 1, 3)
        .reshape(n_cores, 128, NWIN * C)
        .copy()
    )
    cnt_full = np.bincount(t_u, minlength=N).astype(np.float32)
    a_full = np.zeros(n_cores * TPC, np.float32)
    a_full[:N] = missing.astype(np.float32) / np.maximum(cnt_full, 1.0)
    mmask = (
        a_full.reshape(n_cores, NWIN, W).transpose(0, 2, 1).reshape(n_cores, 128, NWIN).copy()
    )

    iota = np.broadcast_to(np.arange(W, dtype=np.float32), (128, W)).astype(BF16).copy()
    tloc_bf = tloc.astype(BF16)

    sched = dict(
        NWIN=NWIN, TPC=TPC, C=C, NP=NP,
        TFA=[int(x) for x in TFA], TFB=[int(x) for x in TFB],
        BT=[int(x) for x in BT],
        a_off=[int(x) for x in a_off], b_off=[int(x) for x in b_off],
        TA=TA, TB=TB,
        t_off=[int(x) for x in t_off],
        NIDXC=NIDXC, SBT=SBT,
    )
    arrays = dict(gidx=gidx, tloc=tloc_bf, x0m=x0m, mmask=mmask, iota=iota)
    return sched, arrays


def _model_numpy(table, sched, arrays, n_cores):
    """Numpy replica of the device computation (for validating prep)."""
    NWIN, C = sched["NWIN"], sched["C"]
    TFA, TFB = sched["TFA"], sched["TFB"]
    g_off, t_off = sched["g_off"], sched["t_off"]
    NP = sched["NP"]
    tb = np.asarray(table, np.float32).astype(BF16).astype(np.float32)
    outs = []
    for c in range(n_cores):
        gidx = arrays["gidx"][c]
        tloc = np.asarray(arrays["tloc"][c], np.float32)
        x0m = arrays["x0m"][c]
        mm = arrays["mmask"][c]
        out = np.zeros((NWIN * W, C), np.float32)
        for w in range(NWIN):
            ntf = TFA[w] + TFB[w]
            bt = ntf
            stag = np.zeros((128, ntf, C), np.float32)
            for half, (nt, coff, base) in enumerate(
                [(TFA[w], g_off[w], 0), (TFB[w], g_off[w] + TFA[w] * 8, HALF)]
            ):
                ni = nt * 128
                if ni == 0:
                    continue
                i = np.arange(ni)
                idx = gidx[i % 16, coff + i // 16].astype(np.int64)
                rows = tb[np.clip(idx + base, 0, NP - 1)]
                t0 = 0 if half == 0 else TFA[w]
                stag[i % 128, t0 + i // 128] = rows
            tl = tloc[:, t_off[w]:t_off[w] + bt]
            oh = (np.arange(W)[None, None, :] == tl[:, :, None]).astype(np.float32)
            feat = np.zeros((W, C), np.float32)
            for t in range(bt):
                feat += oh[:, t, :].T @ stag[:, t, :]
            a = mm[:, w]
            out[w * W:(w + 1) * W] = feat * a[:, None] + x0m[:, w * C:(w + 1) * C]
        outs.append(out)
    return outs


def _build_nc(sched):
    import concourse.bacc as bacc
    import concourse.mybir as mybir
    from concourse import tile

    NWIN, C, NP = sched["NWIN"], sched["C"], sched["NP"]
    TFA, TFB, BT = sched["TFA"], sched["TFB"], sched["BT"]
    a_off, b_off = sched["a_off"], sched["b_off"]
    TA, TB = sched["TA"], sched["TB"]
    t_off = sched["t_off"]
    NIDXC, SBT = sched["NIDXC"], sched["SBT"]
    MAXBT = max(BT)
    f32 = mybir.dt.float32
    bf16 = mybir.dt.bfloat16
    tdt = mybir.dt.float8e4 if USE_FP8 else bf16  # table/stag/one-hot dtype

    nc = bacc.Bacc(None, num_swdge_queues=NQUEUES, dynamic_dma_scratch_size=SCRATCH)
    table_d = nc.dram_tensor("table", [NP, CP], tdt, kind="ExternalInput")
    gidx_d = nc.dram_tensor("gidx", [128, NIDXC], mybir.dt.int16, kind="ExternalInput")
    tloc_d = nc.dram_tensor("tloc", [128, SBT], bf16, kind="ExternalInput")
    iota_d = nc.dram_tensor("iota", [128, W], bf16, kind="ExternalInput")
    mm_d = nc.dram_tensor("mmask", [128, NWIN], f32, kind="ExternalInput")
    x0m_d = nc.dram_tensor("x0m", [128, NWIN * C], f32, kind="ExternalInput")
    out_d = nc.dram_tensor("out", [NWIN * W, C], f32, kind="ExternalOutput")

    tabA = table_d[0:min(HALF, NP), :]
    tabB = table_d[HALF:NP, :] if NP > HALF else None

    # gather-call plan: one call per (stream, GRP-window group). Keeps calls
    # in the ~800-1600 desc sweet spot: big enough to amortize the ~4.4us
    # per-call floor (drain-paced slice + in-flight-2 dispatch), small enough
    # to avoid the outstanding-descriptor stall seen at ~2k+ desc calls.
    # B-half windows carry ~half the tiles of A, so B groups can span more
    # windows for the same call size.
    GRPA = CHUNK_T
    GRPB = int(_os.environ.get("K_GRPB", str(CHUNK_T)))
    GRPS = (GRPA, GRPB)

    def plan(offs, stream):
        chunks = []
        grp = GRPS[stream]
        for w0 in range(0, NWIN, grp):
            w1 = min(w0 + grp, NWIN)
            t0, t1 = offs[w0], offs[w1]
            if t1 > t0:
                chunks.append((stream, t0, t1 - t0, w0))
        return chunks

    # interleave A/B chunk emissions in window order of first use
    emits = {w: [] for w in range(NWIN)}
    for stream, t0, nt, w0 in plan(a_off, 0) + plan(b_off, 1):
        emits[w0].append((stream, t0, nt))

    DEP = int(_os.environ.get("K_DEP", "3"))  # in-flight chunk slots per stream
    qn = [0]

    def next_q(stream):
        q = qn[0] % NQUEUES
        qn[0] += 1
        return q

    with tile.TileContext(nc) as tc:
        with (
            tc.tile_pool(name="const", bufs=1) as cpool,
            tc.tile_pool(name="oh", bufs=4) as opool,
            tc.tile_pool(name="psum", bufs=PSUM_BUFS, space="PSUM") as ppool,
            tc.tile_pool(name="outb", bufs=4) as bpool,
        ):
            GA_MAX = max(a_off[min(w0 + GRPA, NWIN)] - a_off[w0] for w0 in range(0, NWIN, GRPA))
            GB_MAX = max(b_off[min(w0 + GRPB, NWIN)] - b_off[w0] for w0 in range(0, NWIN, GRPB))
            idx_s = cpool.tile([128, NIDXC], mybir.dt.int16)
            tloc_s = cpool.tile([128, SBT], bf16)
            iota_s = cpool.tile([128, W], bf16)
            m_s = cpool.tile([128, NWIN], f32)
            x0m_s = cpool.tile([128, NWIN * C], f32)
            stagA = cpool.tile([128, DEP * GA_MAX * CP], tdt)
            stagB = cpool.tile([128, DEP * max(GB_MAX, 1) * CP], tdt)
            stA = stagA[:].rearrange("p (t c) -> p t c", c=CP)
            stB = stagB[:].rearrange("p (t c) -> p t c", c=CP)
            # split the idx load so the first gather groups' indices land
            # quickly instead of waiting for the full 1.6MB array
            wE = min(4 * GRPA, NWIN)
            cutA = a_off[wE] * 8
            cutB = (TA + b_off[min(4 * GRPB, NWIN)]) * 8
            nc.sync.dma_start(idx_s[:, 0:cutA], gidx_d[:, 0:cutA])
            nc.sync.dma_start(idx_s[:, TA * 8:cutB], gidx_d[:, TA * 8:cutB])
            nc.sync.dma_start(idx_s[:, cutA:TA * 8], gidx_d[:, cutA:TA * 8])
            nc.sync.dma_start(idx_s[:, cutB:NIDXC], gidx_d[:, cutB:NIDXC])
            nc.sync.dma_start(iota_s[:], iota_d[:])
            nc.sync.dma_start(tloc_s[:], tloc_d[:])
            nc.sync.dma_start(m_s[:], mm_d[:])
            nc.sync.dma_start(x0m_s[:], x0m_d[:])
            dummy = None
            if _os.environ.get("K_DIAG_NOCONS"):
                dummy = cpool.tile([128, CP], bf16)
                nc.vector.memset(dummy[:], 0.0)

            def slot_ap(stream, w, g):
                # g = global tile index in the stream; slot by window group
                st = stA if stream == 0 else stB
                gmax = GA_MAX if stream == 0 else max(GB_MAX, 1)
                offs = a_off if stream == 0 else b_off
                grp = GRPS[stream]
                grpi = w // grp
                base = ((grpi % DEP) * gmax) + (g - offs[grpi * grp])
                return st[:, base, :]

            for w in range(NWIN):
                for stream, t0, nt in emits[w]:
                    ni = nt * 128
                    tab = tabA if stream == 0 else tabB
                    coff = t0 * 8 if stream == 0 else (TA + t0) * 8
                    st = stA if stream == 0 else stB
                    gmax = GA_MAX if stream == 0 else max(GB_MAX, 1)
                    base = ((w // GRPS[stream]) % DEP) * gmax
                    nc.gpsimd.dma_gather(
                        st[:, base:base + nt, :], tab,
                        idx_s[:, coff:coff + ni // 16],
                        ni, ni, CP, single_packet=False, queue_num=next_q(stream),
                    )
                bt = BT[w]
                oh = opool.tile([128, MAXBT * W], tdt, tag="oh")
                oh3 = oh[:].rearrange("p (t w) -> p t w", w=W)
                nc.vector.tensor_tensor(
                    oh3[:, 0:bt, :],
                    iota_s[:].unsqueeze(1).broadcast_to([128, bt, W]),
                    tloc_s[:, t_off[w]:t_off[w] + bt].unsqueeze(2).broadcast_to([128, bt, W]),
                    mybir.AluOpType.is_equal,
                )
                psum = ppool.tile([128, C], f32, tag="ps")
                for t in range(bt):
                    if dummy is not None:
                        stile = dummy[:]
                    elif t < TFA[w]:
                        stile = slot_ap(0, w, a_off[w] + t)
                    else:
                        stile = slot_ap(1, w, b_off[w] + t - TFA[w])
                    nc.tensor.matmul(
                        psum[:, 0:C], oh3[:, t, :], stile[:, 0:C],
                        start=(t == 0), stop=(t == bt - 1), skip_group_check=True,
                    )
                outb = bpool.tile([128, C], f32, tag="outb")
                if _os.environ.get("K_EPI_DVE"):
                    nc.vector.tensor_scalar(
                        outb[:], psum[:, 0:C], m_s[:, w:w + 1], None, mybir.AluOpType.mult
                    )
                else:
                    # PSUM read on the (otherwise idle) Activation engine:
                    # outb = psum * a_w
                    nc.scalar.activation(
                        outb[:], psum[:, 0:C], mybir.ActivationFunctionType.Copy,
                        scale=m_s[:, w:w + 1],
                    )
                nc.vector.tensor_tensor(
                    outb[:], outb[:], x0m_s[:, w * C:(w + 1) * C], mybir.AluOpType.add
                )
                nc.sync.dma_start(out_d[w * W:(w + 1) * W, :], outb[:])
    return nc


def _register_ntff_hook():
    """Provide antenv.axon_hooks (absent in this image) so trace=True works."""
    import sys
    import types
    import ctypes
    import contextlib

    try:
        import antenv.axon_hooks  # noqa: F401
        return True
    except ImportError:
        pass
    so_path = "/opt/axon/libaxon_pjrt.so"
    try:
        lib = ctypes.CDLL(so_path)
    except OSError:
        return False
    if not hasattr(lib, "axon_start_nrt_profile"):
        return False
    lib.axon_start_nrt_profile.argtypes = [
        ctypes.POINTER(ctypes.c_int64),
        ctypes.c_size_t,
    ]
    lib.axon_start_nrt_profile.restype = ctypes.c_int64
    lib.axon_stop_nrt_profile.argtypes = [ctypes.c_char_p]
    lib.axon_stop_nrt_profile.restype = ctypes.c_int64

    @contextlib.contextmanager
    def _hook(output_dir, device_ids):
        import jax

        jax.devices()
        if device_ids:
            ids = (ctypes.c_int64 * len(device_ids))(*device_ids)
            rc = lib.axon_start_nrt_profile(ids, len(device_ids))
        else:
            rc = lib.axon_start_nrt_profile(None, 0)
        if rc != 0:
            raise RuntimeError(f"axon_start_nrt_profile rc={rc}")
        try:
            yield
        finally:
            lib.axon_stop_nrt_profile(str(output_dir).encode())

    mod = types.ModuleType("antenv.axon_hooks")
    mod.get_axon_ntff_profile_hook = lambda: _hook
    mod.set_axon_ntff_profile_hook = lambda h: None
    sys.modules["antenv.axon_hooks"] = mod
    return True


def kernel(x_abstract, perm, edge_index, original_num_nodes):
    global LAST_EXEC_NS, LAST_RESULTS
    import os
    from concourse import bass_utils
    from concourse.bass_utils import run_bass_kernel_spmd

    N = int(original_num_nodes)
    n_cores = 8
    x_abstract = np.ascontiguousarray(np.asarray(x_abstract, np.float32))
    sched, arrays = _prep(x_abstract, perm, edge_index, N, n_cores)

    NP = sched["NP"]
    tnp = FP8 if USE_FP8 else BF16
    table_bf = np.zeros((NP, CP), tnp)
    table_bf[:, :x_abstract.shape[1]] = x_abstract.astype(tnp)

    nc = _build_nc(sched)
    nc.finalize()

    in_maps = []
    for c in range(n_cores):
        in_maps.append(
            dict(
                table=table_bf,
                gidx=arrays["gidx"][c],
                tloc=arrays["tloc"][c],
                iota=arrays["iota"],
                mmask=arrays["mmask"][c],
                x0m=arrays["x0m"][c],
            )
        )
    trace = bool(int(os.environ.get("KERNEL_TRACE", "0")))
    if trace:
        trace = _register_ntff_hook()
        bass_utils.upload_artifacts = lambda tmpdir: f"local:{tmpdir}"
    try:
        res = run_bass_kernel_spmd(
            nc, in_maps, core_ids=list(range(n_cores)), trace=trace
        )
    except Exception:
        if not trace:
            raise
        res = run_bass_kernel_spmd(
            nc, in_maps, core_ids=list(range(n_cores)), trace=False
        )
    LAST_RESULTS = res
    LAST_EXEC_NS = getattr(res, "exec_time_ns", None)
    out = np.concatenate([res.results[c]["out"] for c in range(n_cores)], axis=0)
    return out[:N]

